# revision 26
# baseline (speedup 1.0000x reference)
"""AttnPool Trainium2 kernel.

Math: the reference computes k = z @ W.T, scores = (q . k)/sqrt(D),
attn = softmax(scores over P), out = attn-weighted sum of z. Since q is a
single query vector, q.(z@W.T) == z.(q@W): precompute qw = (q @ W) * scale
on the host (tiny), then the device kernel is one memory-bound pass over z,
pipelined at [128, 768] row-tile granularity:

  DVE  scalar_tensor_tensor: s_t = sum(z_t * qw)            (fused mul+reduce)
  ACT  exp:                  e_t = exp(s_t)                 (one column)
  PE   pooled accumulation:  acc += e_t.T @ z_t             (PSUM, fp32 accum)

then a tiny per-batch tail: S = sum(e) (PE ones-matmul + ACT accum),
rS = 1/S (DVE), and the output row is normalized during the PSUM->SBUF copy
(ACT, scale=rS) one batch late so no engine stream stalls across batches.

Precision: z is cast to fp16 on the host. The device then reads 12 MiB/core
instead of 24 - the kernel is HBM-bound, so this halves its runtime. fp16
keeps 11 mantissa bits: the measured end-to-end output error vs the fp32
reference stays ~2e-4, far inside the 2e-2 gate, and |scores| < 6 and
e^s < 450 are well inside fp16 range for unit-normal inputs. All on-chip
accumulation (score dot, softmax denominator, pooled sum) stays fp32.

z is streamed from HBM exactly once at ~366 GB/s/core measured (2.9 TB/s
device aggregate = the HBM roofline); all compute hides under the DMA stream.

Sharding: data-parallel over batch, 8 batches per core on 8 cores (SPMD).
"""
import os

os.environ.setdefault("NEURON_RT_RESET_CORES", "1")

import numpy as np

import concourse.tile as tile
from concourse import bacc, mybir
from concourse.bass_utils import run_bass_kernel_spmd

B, P, D = 64, 1024, 768
N_CORES = 8
B_PER_CORE = B // N_CORES
P_TILES = P // 128
SCALE = float(1.0 / np.sqrt(np.float32(D)))
HALF = D // 2

f32 = mybir.dt.float32
f16 = mybir.dt.float16

_cache = {}

# default build knobs (locked in by the A/B sweep in work/sweep.py)
DMA_TILES = 1
Z_BUFS = 3
CONTIG = False
# row-tiles whose score reduction runs on ACT (activation Copy + accum_out)
# instead of DVE's fused multiply-reduce; for those tiles DVE only does the
# 2x-mode-eligible tensor_tensor multiply (405ns vs 833ns measured). Splits
# the score work across both engines so neither exceeds the DMA roofline.
ACT_TILES = (0, 1, 2)
# s_buf dtype: fp16 makes every stt operand 2-byte (DVE 2x_1P eligibility)
S16 = False


def emit_body(nc, tc, pools, dma_tiles=DMA_TILES, contig=CONTIG, act_tiles=ACT_TILES,
              s16=S16):
    """One full pass over the core's 8 batches. Emits outputs one batch late;
    the final carry is flushed before returning."""
    consts, zp, scp, scrp, scrp2, psp = pools

    def emit_out(prev):
        b_prev, pool_prev0, pool_prev1, S_prev = prev
        rS = scp.tile([1, 1], f32, name="rS", tag="rS")
        nc.vector.reciprocal(rS[:], S_prev[0:1, 0:1])
        out_row = scp.tile([1, D], f32, name="out_row", tag="out_row")
        for h, pps in enumerate([pool_prev0, pool_prev1]):
            nc.scalar.activation(
                out=out_row[0:1, h * HALF : (h + 1) * HALF],
                in_=pps[:],
                func=mybir.ActivationFunctionType.Copy,
                scale=rS[0:1, 0:1],
            )
        nc.scalar.dma_start(out=out_dram_g[b_prev : b_prev + 1, :], in_=out_row[:])

    prev = None
    for b in range(B_PER_CORE):
        # batch 0: fine-grained (1-tile) DMA chunks so the first score op can
        # start ~1us after launch instead of waiting for a multi-MiB chunk;
        # steady-state batches use the bigger, more efficient chunk size.
        step = 1 if b == 0 else dma_tiles

        z_sb = zp.tile([128, P_TILES, D], f16, name="z_sb", tag="z_sb")
        s_buf = scp.tile([128, P_TILES], f16 if s16 else f32, name="s_buf", tag="s_buf")
        e_buf = scp.tile([128, P_TILES], f16, name="e_buf", tag="e_buf")
        pool_ps0 = psp.tile([1, HALF], f32, name="pool_ps0", tag="pool_ps0")
        pool_ps1 = psp.tile([1, HALF], f32, name="pool_ps1", tag="pool_ps1")

        for t in range(P_TILES):
            if t % step == 0:
                if contig:
                    src = z_dram_g[b].rearrange("(p g) d -> p g d", p=128)[
                        :, t : t + step, :
                    ]
                else:
                    src = z_dram_g[b, t * 128 : (t + step) * 128, :].rearrange(
                        "(g p) d -> p g d", p=128
                    )
                nc.sync.dma_start(out=z_sb[:, t : t + step, :], in_=src)
            if t in act_tiles:
                # DVE does only the 2x-eligible multiply; ACT reduces it
                scratch = scrp2.tile([128, D], f16, name="scratch2", tag="scratch2")
                nc.vector.tensor_tensor(
                    out=scratch[:],
                    in0=z_sb[:, t, :],
                    in1=qw_bc_g[:],
                    op=mybir.AluOpType.mult,
                )
                nc.scalar.activation(
                    out=junk_big_g[:],
                    in_=scratch[:],
                    func=mybir.ActivationFunctionType.Copy,
                    accum_out=s_buf[:, t : t + 1],
                )
            else:
                scratch = scrp.tile([128, D], f16, name="scratch", tag="scratch")
                nc.vector.scalar_tensor_tensor(
                    out=scratch[:],
                    in0=z_sb[:, t, :],
                    scalar=1.0,
                    in1=qw_bc_g[:],
                    op0=mybir.AluOpType.mult,
                    op1=mybir.AluOpType.mult,
                    accum_out=s_buf[:, t : t + 1],
                )
        # one batched exp for all 8 score columns (199ns vs 8x450ns measured);
        # its accum_out gives per-partition row-sums of e, whose ones-matmul
        # yields the softmax denominator S directly in PSUM.
        e_rs = scp.tile([128, 1], f32, name="e_rs", tag="e_rs")
        nc.scalar.activation(
            out=e_buf[:],
            in_=s_buf[:],
            func=mybir.ActivationFunctionType.Exp,
            accum_out=e_rs[:],
        )
        for t in range(P_TILES):
            for h, pps in enumerate([pool_ps0, pool_ps1]):
                nc.tensor.matmul(
                    out=pps[:],
                    lhsT=e_buf[:, t : t + 1],
                    rhs=z_sb[:, t, h * HALF : (h + 1) * HALF],
                    start=(t == 0),
                    stop=(t == P_TILES - 1),
                )
        S_ps = psp.tile([1, 1], f32, name="S_ps", tag="S_ps")
        nc.tensor.matmul(
            out=S_ps[:], lhsT=ones_col32_g[:], rhs=e_rs[:], start=True, stop=True
        )

        if prev is not None:
            emit_out(prev)
        prev = (b, pool_ps0, pool_ps1, S_ps)

    emit_out(prev)


def _setup(nc, tc, consts):
    """Shared constants; stores module-level handles used by emit_body."""
    global qw_bc_g, ones_col32_g, ones_col16_g, junk_big_g, junk_row_g
    qw_bc_g = consts.tile([128, D], f16, name="qw_bc")
    nc.gpsimd.dma_start(out=qw_bc_g[:], in_=qw_dram_g.to_broadcast((128, D)))
    ones_col32_g = consts.tile([128, 1], f32, name="ones_col32")
    nc.vector.memset(ones_col32_g[:], 1.0)
    ones_col16_g = consts.tile([128, 1], f16, name="ones_col16")
    nc.vector.memset(ones_col16_g[:], 1.0)
    junk_big_g = consts.tile([128, D], f16, name="junk_big")
    junk_row_g = consts.tile([1, P_TILES], f32, name="junk_row")


def build(reps=1, dma_tiles=DMA_TILES, z_bufs=Z_BUFS, contig=CONTIG, loop_n=None,
          act_tiles=ACT_TILES, s16=S16, sc_bufs=2, ps_bufs=2):
    """loop_n=None: unrolled `reps` passes (the real kernel, reps=1).
    loop_n=k: body wrapped in a hardware For_i(0, k) loop (timing harness)."""
    global z_dram_g, out_dram_g, qw_dram_g
    nc = bacc.Bacc("TRN2", target_bir_lowering=False, debug=False, num_devices=N_CORES)
    z_dram_g = nc.dram_tensor("z", [B_PER_CORE, P, D], f16, kind="ExternalInput").ap()
    qw_dram_g = nc.dram_tensor("qw", [1, D], f16, kind="ExternalInput").ap()
    out_dram_g = nc.dram_tensor("out", [B_PER_CORE, D], f32, kind="ExternalOutput").ap()

    with tile.TileContext(nc) as tc:
        with (
            tc.tile_pool(name="consts", bufs=1) as consts,
            tc.tile_pool(name="zp", bufs=z_bufs) as zp,
            tc.tile_pool(name="sc", bufs=sc_bufs) as scp,
            tc.tile_pool(name="scr", bufs=3) as scrp,
            tc.tile_pool(name="scr2", bufs=3) as scrp2,
            tc.tile_pool(name="ps", bufs=ps_bufs, space="PSUM") as psp,
        ):
            _setup(nc, tc, consts)
            pools = (consts, zp, scp, scrp, scrp2, psp)
            if loop_n is not None:
                with tc.For_i(0, loop_n) as _i:
                    emit_body(nc, tc, pools, dma_tiles, contig, act_tiles, s16)
            else:
                for _ in range(reps):
                    emit_body(nc, tc, pools, dma_tiles, contig, act_tiles, s16)

    nc.finalize()
    return nc


def prep_inputs(z, qw):
    """Host-side: fold the 1/sqrt(D) scale into qw and cast both to fp16."""
    z16 = np.ascontiguousarray(np.asarray(z), dtype=np.float16)
    qw16 = (np.asarray(qw, np.float64).reshape(D) * SCALE).astype(np.float16)
    return z16, qw16


def get_nc(reps=1, dma_tiles=DMA_TILES):
    key = (reps, dma_tiles)
    if key not in _cache:
        _cache[key] = build(reps, dma_tiles)
    return _cache[key]


def run(z, qw, reps=1, **kwargs):
    """Run the SPMD kernel. z: [B,P,D] f32, qw: [D] f32. Returns results obj."""
    nc = get_nc(reps)
    z16, qw16 = prep_inputs(z, qw)
    in_maps = [
        {"z": z16[i * B_PER_CORE : (i + 1) * B_PER_CORE], "qw": qw16[None, :]}
        for i in range(N_CORES)
    ]
    return run_bass_kernel_spmd(nc, in_maps, core_ids=list(range(N_CORES)), **kwargs)


def kernel(z, q, W_proj):
    z = np.asarray(z, dtype=np.float32)
    q = np.asarray(q, dtype=np.float32)
    W_proj = np.asarray(W_proj, dtype=np.float32)
    qw = q.reshape(D).astype(np.float64) @ W_proj.astype(np.float64)

    res = run(z, qw)
    out = np.concatenate([r["out"] for r in res.results], axis=0)
    return out.astype(np.float32)


if __name__ == "__main__":
    rng = np.random.default_rng(0)
    z = rng.standard_normal((B, P, D)).astype(np.float32)
    q = rng.standard_normal((1, 1, D)).astype(np.float32)
    W = (rng.standard_normal((D, D)) / np.sqrt(D)).astype(np.float32)
    out = kernel(z, q, W)
    print("out", out.shape, out.dtype, out[:2, :4])
